# revision 3
# baseline (speedup 1.0000x reference)
"""Trainium2 Bass kernel for nn_DiffeqSolver (RK4 ODE solve, 2-layer tanh MLP drift).

Strategy (data-parallel across 8 NeuronCores):
  - Shard the 32768 latent rows (NTRAJ*B*N) across 8 cores -> 4096 rows/core.
  - Everything on-chip is feature-major ("transposed"): y^T [64, rows], so both
    matmuls of the MLP have their contraction dim on SBUF partitions.
  - Per RK4 stage i: z = W1^T y_i^T (2 matmuls, M-blocks of 128, into one merged
    PSUM tile [128, 2, 512]), a = tanh(z) (single wide ACT op), then
    P_i = (s_i W2)^T a (2 accumulating matmuls, K-blocks of 128) where the RK4
    step-size factors s_i in (h/2, h/2, h, h/6) are folded into host-prescaled
    copies of W2.  Stage states: y_{i+1} = y + P_i (one fused DVE op).
    Final combine: y_next = (y2 + 2*y3 + y4 - y)/3 + P4 (GpSimd+DVE fused ops).
  - Output is written transposed ([steps, 64, rows] per core); the host
    re-transposes while gathering.
"""

import sys

if "/opt/trn_rl_repo" not in sys.path:
    sys.path.insert(0, "/opt/trn_rl_repo")

import numpy as np

_NCORES = 8
_T = 32
_NTRAJ, _B, _N, _L = 1, 32, 1024, 64
_H = 256
_ROWS = _NTRAJ * _B * _N          # 32768 total latent rows
_R = _ROWS // _NCORES             # 4096 rows per core
_WT = 512                         # column-tile width (matmul moving-dim)
_NT = _R // _WT                   # 8 column tiles per core

_BUILD_CACHE = {}


def _build(nsteps: int, n_hslots: int, b1_nonzero: bool, b2_nonzero: bool):
    import concourse.mybir as mybir
    import concourse.tile as tile
    from concourse import bacc

    f32 = mybir.dt.float32
    Alu = mybir.AluOpType
    Act = mybir.ActivationFunctionType

    nc = bacc.Bacc("TRN2", target_bir_lowering=False, debug=False,
                   num_devices=_NCORES)

    y0t = nc.dram_tensor("y0t", [_L, _R], f32, kind="ExternalInput")
    w1d = nc.dram_tensor("w1d", [_L, _H], f32, kind="ExternalInput")
    # Host-prescaled W2 variants: [128, slot, variant(h/2, h, h/6), kblock, 64]
    w2d = nc.dram_tensor("w2d", [128, n_hslots, 3, 2, _L], f32,
                         kind="ExternalInput")
    b1d = (nc.dram_tensor("b1d", [128, 2], f32, kind="ExternalInput")
           if b1_nonzero else None)
    # b2 scaled by s_i per variant, plus a 4th column holding 3*(h/6)*b2
    b2d = (nc.dram_tensor("b2d", [_L, n_hslots, 4], f32, kind="ExternalInput")
           if b2_nonzero else None)
    outt = nc.dram_tensor("outt", [nsteps, _L, _R], f32, kind="ExternalOutput")

    with tile.TileContext(nc) as tc:
        with (
            tc.tile_pool(name="singles", bufs=1) as singles,
            tc.tile_pool(name="zpool", bufs=3, space="PSUM") as zpool,
            tc.tile_pool(name="ppool", bufs=2, space="PSUM") as ppool,
            tc.tile_pool(name="apool", bufs=3) as apool,
            tc.tile_pool(name="ypool", bufs=8) as ypool,
            tc.tile_pool(name="gpool", bufs=4) as gpool,
        ):
            ybuf = [singles.tile([_L, _R], f32, tag="ybuf0", name="ybuf0"),
                    singles.tile([_L, _R], f32, tag="ybuf1", name="ybuf1")]
            w1sb = singles.tile([_L, _H], f32, tag="w1sb")
            w2sb = singles.tile([128, n_hslots, 3, 2, _L], f32, tag="w2sb")
            nc.sync.dma_start(out=ybuf[0][:, :], in_=y0t.ap())
            nc.sync.dma_start(out=w1sb[:, :], in_=w1d.ap())
            nc.sync.dma_start(out=w2sb[:, :, :, :, :], in_=w2d.ap())
            if b1_nonzero:
                b1sb = singles.tile([128, 2], f32, tag="b1sb")
                nc.sync.dma_start(out=b1sb[:, :], in_=b1d.ap())
            if b2_nonzero:
                b2sb = singles.tile([_L, n_hslots, 4], f32, tag="b2sb")
                nc.sync.dma_start(out=b2sb[:, :, :], in_=b2d.ap())

            for s in range(nsteps):
                slot = 0 if n_hslots == 1 else s
                ycur = ybuf[s % 2]
                ynxt = ybuf[(s + 1) % 2]
                for t in range(_NT):
                    ysl = ycur[:, t * _WT:(t + 1) * _WT]
                    prev = ysl
                    ystage = []
                    for e in range(4):
                        v = 0 if e < 2 else (1 if e == 2 else 2)
                        z = zpool.tile([128, 2, _WT], f32, tag="z")
                        nc.tensor.matmul(z[:, 0], w1sb[:, 0:128], prev,
                                         start=True, stop=True)
                        nc.tensor.matmul(z[:, 1], w1sb[:, 128:256], prev,
                                         start=True, stop=True)
                        a = apool.tile([128, 2, _WT], f32, tag="a")
                        if b1_nonzero:
                            nc.scalar.activation(a[:, 0], z[:, 0], Act.Tanh,
                                                 bias=b1sb[:, 0])
                            nc.scalar.activation(a[:, 1], z[:, 1], Act.Tanh,
                                                 bias=b1sb[:, 1])
                        else:
                            nc.scalar.activation(a[:, :, :], z[:, :, :],
                                                 Act.Tanh)
                        p = ppool.tile([_L, _WT], f32, tag="p")
                        nc.tensor.matmul(p[:, :], w2sb[:, slot, v, 0], a[:, 0],
                                         start=True, stop=False)
                        nc.tensor.matmul(p[:, :], w2sb[:, slot, v, 1], a[:, 1],
                                         start=False, stop=True)
                        if e < 3:
                            yn = ypool.tile([_L, _WT], f32, tag="yn")
                            if b2_nonzero:
                                nc.vector.scalar_tensor_tensor(
                                    yn[:, :], p[:, :], b2sb[:, slot, v],
                                    ysl, Alu.add, Alu.add)
                            else:
                                nc.vector.tensor_add(yn[:, :], p[:, :], ysl)
                            ystage.append(yn)
                            prev = yn[:, :]
                        else:
                            y2v, y3v, y4v = ystage
                            g0 = gpool.tile([_L, _WT], f32, tag="g0")
                            nc.gpsimd.tensor_scalar_mul(g0[:, :], y3v[:, :],
                                                        2.0)
                            g1 = gpool.tile([_L, _WT], f32, tag="g1")
                            nc.gpsimd.tensor_add(g1[:, :], g0[:, :],
                                                 y2v[:, :])
                            g2 = gpool.tile([_L, _WT], f32, tag="g2")
                            if b2_nonzero:
                                nc.vector.scalar_tensor_tensor(
                                    g2[:, :], y4v[:, :], b2sb[:, slot, 3],
                                    g1[:, :], Alu.add, Alu.add)
                            else:
                                nc.vector.tensor_add(g2[:, :], g1[:, :],
                                                     y4v[:, :])
                            d1 = gpool.tile([_L, _WT], f32, tag="d1")
                            nc.vector.tensor_sub(d1[:, :], g2[:, :], ysl)
                            nc.vector.scalar_tensor_tensor(
                                ynxt[:, t * _WT:(t + 1) * _WT], d1[:, :],
                                1.0 / 3.0, p[:, :], Alu.mult, Alu.add)
                nc.sync.dma_start(out=outt.ap()[s], in_=ynxt[:, :])

    nc.compile()
    return nc


def _prep_inputs(first_point, time_steps_to_predict, W1, b1, W2, b2):
    """Host-side shard + transpose + weight prescale. Returns (key, in_maps, nsteps)."""
    fp = np.ascontiguousarray(np.asarray(first_point, dtype=np.float32))
    ts = np.asarray(time_steps_to_predict, dtype=np.float32)
    W1 = np.ascontiguousarray(np.asarray(W1, dtype=np.float32))
    W2 = np.ascontiguousarray(np.asarray(W2, dtype=np.float32))
    b1 = np.asarray(b1, dtype=np.float32)
    b2 = np.asarray(b2, dtype=np.float32)

    nsteps = int(ts.shape[0]) - 1
    hs = np.diff(ts.astype(np.float64)).astype(np.float32)      # [nsteps]
    uniform = bool(np.all(hs == hs[0]))
    n_hslots = 1 if uniform else nsteps
    hs_used = hs[:1] if uniform else hs

    b1_nonzero = bool(np.any(b1))
    b2_nonzero = bool(np.any(b2))

    flat = fp.reshape(_ROWS, _L)

    # W2 as [128 partitions, kblock, 64], scaled per (slot, variant)
    w2kb = W2.reshape(2, 128, _L).transpose(1, 0, 2)            # [128, 2, 64]
    scales = np.stack([hs_used / 2.0, hs_used, hs_used / 6.0], axis=1)  # [S,3]
    w2s = (scales[None, :, :, None, None] *
           w2kb[:, None, None, :, :]).astype(np.float32)        # [128,S,3,2,64]
    w2s = np.ascontiguousarray(w2s)

    in_maps = []
    for c in range(_NCORES):
        shard = flat[c * _R:(c + 1) * _R]                       # [R, 64]
        y0t = np.ascontiguousarray(shard.T)                     # [64, R]
        m = {"y0t": y0t, "w1d": W1, "w2d": w2s}
        if b1_nonzero:
            m["b1d"] = np.ascontiguousarray(b1.reshape(2, 128).T)
        if b2_nonzero:
            b2s = np.empty((_L, n_hslots, 4), np.float32)
            b2s[:, :, 0] = b2[:, None] * (hs_used / 2.0)[None, :]
            b2s[:, :, 1] = b2[:, None] * hs_used[None, :]
            b2s[:, :, 2] = b2[:, None] * (hs_used / 6.0)[None, :]
            b2s[:, :, 3] = b2[:, None] * (hs_used / 2.0)[None, :]  # 3*(h/6)
            m["b2d"] = b2s
        in_maps.append(m)

    key = (nsteps, n_hslots, b1_nonzero, b2_nonzero)
    return key, in_maps, nsteps


def get_nc(first_point, time_steps_to_predict, W1, b1, W2, b2):
    """Build (or fetch cached) the compiled Bass program for these inputs."""
    key, in_maps, nsteps = _prep_inputs(
        first_point, time_steps_to_predict, W1, b1, W2, b2)
    if key not in _BUILD_CACHE:
        _BUILD_CACHE[key] = _build(*key)
    return _BUILD_CACHE[key], in_maps, nsteps


def _assemble(first_point, core_outs, nsteps):
    """core_outs: list of [nsteps, 64, R] per core -> full [1, T, B, N, L]."""
    fp = np.asarray(first_point, dtype=np.float32)
    out = np.empty((_NTRAJ, nsteps + 1, _B, _N, _L), np.float32)
    out[:, 0] = fp
    bs = _B // _NCORES                                          # batches/core
    for c in range(_NCORES):
        dev = core_outs[c]                                      # [S, 64, R]
        # -> [S, R, 64] -> [S, bs, N, L]
        out[0, 1:, c * bs:(c + 1) * bs] = dev.transpose(0, 2, 1).reshape(
            nsteps, bs, _N, _L)
    return out


def kernel(first_point, time_steps_to_predict, W1, b1, W2, b2):
    from concourse.bass_utils import run_bass_kernel_spmd

    nc, in_maps, nsteps = get_nc(
        first_point, time_steps_to_predict, W1, b1, W2, b2)
    res = run_bass_kernel_spmd(nc, in_maps, core_ids=list(range(_NCORES)))
    core_outs = [res.results[c]["outt"] for c in range(_NCORES)]
    return _assemble(first_point, core_outs, nsteps)


# revision 4
# speedup vs baseline: 1.0070x; 1.0070x over previous
"""Trainium2 Bass kernel for nn_DiffeqSolver (RK4 ODE solve, 2-layer tanh MLP drift).

Strategy (data-parallel across 8 NeuronCores):
  - Shard the 32768 latent rows (NTRAJ*B*N) across 8 cores -> 4096 rows/core.
  - Everything on-chip is feature-major ("transposed"): y^T [64, rows], so both
    matmuls of the MLP have their contraction dim on SBUF partitions.
  - Per RK4 stage i: z = W1^T y_i^T (2 matmuls, M-blocks of 128, into one merged
    PSUM tile [128, 2, 512]), a = tanh(z) (single wide ACT op), then
    P_i = (s_i W2)^T a (2 accumulating matmuls, K-blocks of 128) where the RK4
    step-size factors s_i in (h/2, h/2, h, h/6) are folded into host-prescaled
    copies of W2.  Stage states: y_{i+1} = y + P_i (one fused DVE op).
    Final combine: y_next = (y2 + 2*y3 + y4 - y)/3 + P4 (GpSimd+DVE fused ops).
  - Output is written transposed ([steps, 64, rows] per core); the host
    re-transposes while gathering.
"""

import sys

if "/opt/trn_rl_repo" not in sys.path:
    sys.path.insert(0, "/opt/trn_rl_repo")

import numpy as np

_NCORES = 8
_T = 32
_NTRAJ, _B, _N, _L = 1, 32, 1024, 64
_H = 256
_ROWS = _NTRAJ * _B * _N          # 32768 total latent rows
_R = _ROWS // _NCORES             # 4096 rows per core
_WT = 512                         # column-tile width (matmul moving-dim)
_NT = _R // _WT                   # 8 column tiles per core

_BUILD_CACHE = {}


def _build(nsteps: int, n_hslots: int, b1_nonzero: bool, b2_nonzero: bool):
    import concourse.mybir as mybir
    import concourse.tile as tile
    from concourse import bacc

    f32 = mybir.dt.float32
    f32r = mybir.dt.float32r
    Alu = mybir.AluOpType
    Act = mybir.ActivationFunctionType

    nc = bacc.Bacc("TRN2", target_bir_lowering=False, debug=False,
                   num_devices=_NCORES)

    y0t = nc.dram_tensor("y0t", [_L, _R], f32r, kind="ExternalInput")
    w1d = nc.dram_tensor("w1d", [_L, _H], f32r, kind="ExternalInput")
    # Host-prescaled W2 variants: [128, slot, variant(h/2, h, h/6), kblock, 64]
    w2d = nc.dram_tensor("w2d", [128, n_hslots, 3, 2, _L], f32r,
                         kind="ExternalInput")
    b1d = (nc.dram_tensor("b1d", [128, 2], f32, kind="ExternalInput")
           if b1_nonzero else None)
    # b2 scaled by s_i per variant, plus a 4th column holding 3*(h/6)*b2
    b2d = (nc.dram_tensor("b2d", [_L, n_hslots, 4], f32, kind="ExternalInput")
           if b2_nonzero else None)
    outt = nc.dram_tensor("outt", [nsteps, _L, _R], f32r, kind="ExternalOutput")

    with tile.TileContext(nc) as tc:
        with (
            tc.tile_pool(name="singles", bufs=1) as singles,
            tc.tile_pool(name="zpool", bufs=3, space="PSUM") as zpool,
            tc.tile_pool(name="ppool", bufs=2, space="PSUM") as ppool,
            tc.tile_pool(name="apool", bufs=3) as apool,
            tc.tile_pool(name="ypool", bufs=8) as ypool,
            tc.tile_pool(name="gpool", bufs=4) as gpool,
        ):
            ybuf = [singles.tile([_L, _R], f32r, tag="ybuf0", name="ybuf0"),
                    singles.tile([_L, _R], f32r, tag="ybuf1", name="ybuf1")]
            w1sb = singles.tile([_L, _H], f32r, tag="w1sb")
            w2sb = singles.tile([128, n_hslots, 3, 2, _L], f32r, tag="w2sb")
            nc.sync.dma_start(out=ybuf[0][:, :], in_=y0t.ap())
            nc.sync.dma_start(out=w1sb[:, :], in_=w1d.ap())
            nc.sync.dma_start(out=w2sb[:, :, :, :, :], in_=w2d.ap())
            if b1_nonzero:
                b1sb = singles.tile([128, 2], f32, tag="b1sb")
                nc.sync.dma_start(out=b1sb[:, :], in_=b1d.ap())
            if b2_nonzero:
                b2sb = singles.tile([_L, n_hslots, 4], f32, tag="b2sb")
                nc.sync.dma_start(out=b2sb[:, :, :], in_=b2d.ap())

            for s in range(nsteps):
                slot = 0 if n_hslots == 1 else s
                ycur = ybuf[s % 2]
                ynxt = ybuf[(s + 1) % 2]
                for t in range(_NT):
                    ysl = ycur[:, t * _WT:(t + 1) * _WT]
                    prev = ysl
                    ystage = []
                    for e in range(4):
                        v = 0 if e < 2 else (1 if e == 2 else 2)
                        z = zpool.tile([128, 2, _WT], f32, tag="z")
                        nc.tensor.matmul(z[:, 0], w1sb[:, 0:128], prev,
                                         start=True, stop=True)
                        nc.tensor.matmul(z[:, 1], w1sb[:, 128:256], prev,
                                         start=True, stop=True)
                        a = apool.tile([128, 2, _WT], f32r, tag="a")
                        if b1_nonzero:
                            nc.scalar.activation(a[:, 0], z[:, 0], Act.Tanh,
                                                 bias=b1sb[:, 0])
                            nc.scalar.activation(a[:, 1], z[:, 1], Act.Tanh,
                                                 bias=b1sb[:, 1])
                        else:
                            nc.scalar.activation(a[:, :, :], z[:, :, :],
                                                 Act.Tanh)
                        p = ppool.tile([_L, _WT], f32, tag="p")
                        nc.tensor.matmul(p[:, :], w2sb[:, slot, v, 0], a[:, 0],
                                         start=True, stop=False)
                        nc.tensor.matmul(p[:, :], w2sb[:, slot, v, 1], a[:, 1],
                                         start=False, stop=True)
                        if e < 3:
                            yn = ypool.tile([_L, _WT], f32r, tag="yn")
                            if b2_nonzero:
                                nc.vector.scalar_tensor_tensor(
                                    yn[:, :], p[:, :], b2sb[:, slot, v],
                                    ysl, Alu.add, Alu.add)
                            else:
                                nc.vector.tensor_add(yn[:, :], p[:, :], ysl)
                            ystage.append(yn)
                            prev = yn[:, :]
                        else:
                            y2v, y3v, y4v = ystage
                            g0 = gpool.tile([_L, _WT], f32, tag="g0")
                            nc.gpsimd.tensor_scalar_mul(g0[:, :], y3v[:, :],
                                                        2.0)
                            g1 = gpool.tile([_L, _WT], f32, tag="g1")
                            nc.gpsimd.tensor_add(g1[:, :], g0[:, :],
                                                 y2v[:, :])
                            g2 = gpool.tile([_L, _WT], f32, tag="g2")
                            if b2_nonzero:
                                nc.vector.scalar_tensor_tensor(
                                    g2[:, :], y4v[:, :], b2sb[:, slot, 3],
                                    g1[:, :], Alu.add, Alu.add)
                            else:
                                nc.vector.tensor_add(g2[:, :], g1[:, :],
                                                     y4v[:, :])
                            d1 = gpool.tile([_L, _WT], f32, tag="d1")
                            nc.vector.tensor_sub(d1[:, :], g2[:, :], ysl)
                            nc.vector.scalar_tensor_tensor(
                                ynxt[:, t * _WT:(t + 1) * _WT], d1[:, :],
                                1.0 / 3.0, p[:, :], Alu.mult, Alu.add)
                nc.sync.dma_start(out=outt.ap()[s], in_=ynxt[:, :])

    nc.compile()
    return nc


def _prep_inputs(first_point, time_steps_to_predict, W1, b1, W2, b2):
    """Host-side shard + transpose + weight prescale. Returns (key, in_maps, nsteps)."""
    fp = np.ascontiguousarray(np.asarray(first_point, dtype=np.float32))
    ts = np.asarray(time_steps_to_predict, dtype=np.float32)
    W1 = np.ascontiguousarray(np.asarray(W1, dtype=np.float32))
    W2 = np.ascontiguousarray(np.asarray(W2, dtype=np.float32))
    b1 = np.asarray(b1, dtype=np.float32)
    b2 = np.asarray(b2, dtype=np.float32)

    nsteps = int(ts.shape[0]) - 1
    hs = np.diff(ts.astype(np.float64)).astype(np.float32)      # [nsteps]
    uniform = bool(np.all(hs == hs[0]))
    n_hslots = 1 if uniform else nsteps
    hs_used = hs[:1] if uniform else hs

    b1_nonzero = bool(np.any(b1))
    b2_nonzero = bool(np.any(b2))

    flat = fp.reshape(_ROWS, _L)

    # W2 as [128 partitions, kblock, 64], scaled per (slot, variant)
    w2kb = W2.reshape(2, 128, _L).transpose(1, 0, 2)            # [128, 2, 64]
    scales = np.stack([hs_used / 2.0, hs_used, hs_used / 6.0], axis=1)  # [S,3]
    w2s = (scales[None, :, :, None, None] *
           w2kb[:, None, None, :, :]).astype(np.float32)        # [128,S,3,2,64]
    w2s = np.ascontiguousarray(w2s)

    in_maps = []
    for c in range(_NCORES):
        shard = flat[c * _R:(c + 1) * _R]                       # [R, 64]
        y0t = np.ascontiguousarray(shard.T)                     # [64, R]
        m = {"y0t": y0t, "w1d": W1, "w2d": w2s}
        if b1_nonzero:
            m["b1d"] = np.ascontiguousarray(b1.reshape(2, 128).T)
        if b2_nonzero:
            b2s = np.empty((_L, n_hslots, 4), np.float32)
            b2s[:, :, 0] = b2[:, None] * (hs_used / 2.0)[None, :]
            b2s[:, :, 1] = b2[:, None] * hs_used[None, :]
            b2s[:, :, 2] = b2[:, None] * (hs_used / 6.0)[None, :]
            b2s[:, :, 3] = b2[:, None] * (hs_used / 2.0)[None, :]  # 3*(h/6)
            m["b2d"] = b2s
        in_maps.append(m)

    key = (nsteps, n_hslots, b1_nonzero, b2_nonzero)
    return key, in_maps, nsteps


def get_nc(first_point, time_steps_to_predict, W1, b1, W2, b2):
    """Build (or fetch cached) the compiled Bass program for these inputs."""
    key, in_maps, nsteps = _prep_inputs(
        first_point, time_steps_to_predict, W1, b1, W2, b2)
    if key not in _BUILD_CACHE:
        _BUILD_CACHE[key] = _build(*key)
    return _BUILD_CACHE[key], in_maps, nsteps


def _assemble(first_point, core_outs, nsteps):
    """core_outs: list of [nsteps, 64, R] per core -> full [1, T, B, N, L]."""
    fp = np.asarray(first_point, dtype=np.float32)
    out = np.empty((_NTRAJ, nsteps + 1, _B, _N, _L), np.float32)
    out[:, 0] = fp
    bs = _B // _NCORES                                          # batches/core
    for c in range(_NCORES):
        dev = core_outs[c]                                      # [S, 64, R]
        # -> [S, R, 64] -> [S, bs, N, L]
        out[0, 1:, c * bs:(c + 1) * bs] = dev.transpose(0, 2, 1).reshape(
            nsteps, bs, _N, _L)
    return out


def kernel(first_point, time_steps_to_predict, W1, b1, W2, b2):
    from concourse.bass_utils import run_bass_kernel_spmd

    nc, in_maps, nsteps = get_nc(
        first_point, time_steps_to_predict, W1, b1, W2, b2)
    res = run_bass_kernel_spmd(nc, in_maps, core_ids=list(range(_NCORES)))
    core_outs = [res.results[c]["outt"] for c in range(_NCORES)]
    return _assemble(first_point, core_outs, nsteps)
